# revision 2
# baseline (speedup 1.0000x reference)
"""FP8 MoE (top-2, 8 experts) Trainium2 kernel.

Strategy (expert-parallel over 8 NeuronCores):
  - Host: compute per-token per-expert gate = sum(routing_weights where
    selected_experts == e).  Tokens with gate == 0 contribute exactly 0 to the
    reference output, so each expert only processes its selected tokens
    (~T*K/E each instead of T).
  - Host: quantize activations x -> fp8 e4m3fn exactly as the reference does,
    then map the e4m3fn grid onto Trainium's IEEE e4m3 grid (max 240 vs 448)
    by halving (exact: exponent decrement).  Weights (already e4m3fn values
    stored as f32) are halved the same way.  The compensating 4x lands in the
    dequant scales.
  - Device (per core = per expert): h~ = xq_t @ w1_t^T via fp8 DoubleRow
    matmuls (features on PSUM partitions, tokens on the moving free dim);
    aq = fp8(clip(silu(s1*g~) * c2*u~, +-224)); y~ = aq_t @ w2_t^T.
  - Host: out[tok] += gate * s2 * y~  (s2 = 4*iscale2*wscale2), experts in
    ascending order like the reference loop.
"""

import os

import numpy as np
import ml_dtypes

import concourse.mybir as mybir
from concourse import bacc
from concourse.tile import TileContext
from concourse.bass_utils import run_bass_kernel_spmd

FP8_MAX = 448.0
E4 = mybir.dt.float8e4
F32 = mybir.dt.float32
E4NP = ml_dtypes.float8_e4m3        # TRN fp8 (IEEE, max 240)
FNNP = ml_dtypes.float8_e4m3fn      # OCP fp8 (max 448) — the reference format

# Problem sizes (hardcoded; harness contract).
T, H, I, E, TOPK = 4096, 2048, 4096, 8, 2

# Module global holding the most recent BassKernelResults (for test harness).
LAST_RESULT = None


# --------------------------------------------------------------------------
# Device kernel builder (shape-generic so it can be smoke-tested small).
# --------------------------------------------------------------------------

def build_nc(C, NT, h=H, i_dim=I, n_cores=8):
    """Two fp8 GEMMs + SiLU-gate epilogue for one expert over C tokens.

    Inputs (per core):
      xq  [128, 2*KT1, C] fp8e4 : xq^T tiled (k-chunk, pair, token)
      w1p [M1, 128, KT1*256] fp8e4 : w1^T tiled per output-feature block
      w2p [M2, 128, KT2*256] fp8e4
      sc  [128, 2] f32 : col0 = s1 = 4*is1*ws1, col1 = c2 = s1/(2*is2)
    Output:
      yT  [h, C] f32 : y~^T (caller applies s2 = 4*is2*ws2 and the gate)
    """
    assert h % 256 == 0 and i_dim % 256 == 0 and C % NT == 0
    KT1 = h // 256          # mm1 k-chunks (256 deep each w/ DoubleRow)
    KT2 = i_dim // 256      # mm2 k-chunks
    MG = i_dim // 128       # gate-half feature tiles (m and m+MG pair up)
    M1 = 2 * MG             # w1 output tiles
    M2 = h // 128           # w2 output tiles
    NTILES = C // NT
    DR = mybir.MatmulPerfMode.DoubleRow
    mult = mybir.AluOpType.mult

    nc = bacc.Bacc("TRN2", target_bir_lowering=False, debug=False,
                   num_devices=n_cores)
    xq = nc.dram_tensor("xq", [128, 2 * KT1, C], E4, kind="ExternalInput")
    w1p = nc.dram_tensor("w1p", [M1, 128, KT1 * 256], E4, kind="ExternalInput")
    w2p = nc.dram_tensor("w2p", [M2, 128, KT2 * 256], E4, kind="ExternalInput")
    sc = nc.dram_tensor("sc", [128, 2], F32, kind="ExternalInput")
    yT = nc.dram_tensor("yT", [h, C], F32, kind="ExternalOutput")

    with TileContext(nc) as tc:
        with (
            tc.tile_pool(name="cpool", bufs=1) as cpool,
            tc.tile_pool(name="xqpool", bufs=1) as xqpool,
            tc.tile_pool(name="aqpool", bufs=1) as aqpool,
            tc.tile_pool(name="w1pool", bufs=4) as w1pool,
            tc.tile_pool(name="w2pool", bufs=3) as w2pool,
            tc.tile_pool(name="eppool", bufs=3) as eppool,
            tc.tile_pool(name="ypool", bufs=4) as ypool,
            tc.tile_pool(name="psA", bufs=2, space="PSUM") as psA,
            tc.tile_pool(name="psB", bufs=2, space="PSUM") as psB,
            tc.tile_pool(name="psY", bufs=4, space="PSUM") as psY,
        ):
            sc_t = cpool.tile([128, 2], F32, name="sc_t")
            nc.sync.dma_start(out=sc_t, in_=sc.ap())
            s1_ap = sc_t[:, 0:1]
            c2_ap = sc_t[:, 1:2]

            xq_t = xqpool.tile([128, 2 * KT1, C], E4, name="xq_t")
            nc.sync.dma_start(out=xq_t, in_=xq.ap())
            aq_t = aqpool.tile([128, 2 * KT2, C], E4, name="aq_t")

            # ---- mm1 + gated epilogue: aq^T[i_dim, C] in fp8 ----
            for jg in range(MG):
                wg = w1pool.tile([128, KT1 * 256], E4, name="wg", tag="w1")
                nc.sync.dma_start(out=wg, in_=w1p.ap()[jg])
                wu = w1pool.tile([128, KT1 * 256], E4, name="wu", tag="w1")
                nc.sync.dma_start(out=wu, in_=w1p.ap()[jg + MG])
                for nt in range(NTILES):
                    nsl = slice(nt * NT, (nt + 1) * NT)
                    pg = psA.tile([128, NT], F32, name="pg")
                    pu = psB.tile([128, NT], F32, name="pu")
                    for k in range(KT1):
                        lg = wg[:, k * 256:(k + 1) * 256].rearrange(
                            "p (i m) -> p i m", i=2)
                        rx = xq_t[:, 2 * k:2 * k + 2, nsl]
                        nc.tensor.matmul(pg, lg, rx, start=(k == 0),
                                         stop=(k == KT1 - 1), perf_mode=DR)
                    for k in range(KT1):
                        lu = wu[:, k * 256:(k + 1) * 256].rearrange(
                            "p (i m) -> p i m", i=2)
                        rx = xq_t[:, 2 * k:2 * k + 2, nsl]
                        nc.tensor.matmul(pu, lu, rx, start=(k == 0),
                                         stop=(k == KT1 - 1), perf_mode=DR)
                    tg = eppool.tile([128, NT], F32, name="tg", tag="tg")
                    nc.scalar.activation(tg, pg,
                                         mybir.ActivationFunctionType.Silu,
                                         scale=s1_ap)
                    v = eppool.tile([128, NT], F32, name="v", tag="v")
                    nc.vector.scalar_tensor_tensor(v, pu, c2_ap, tg,
                                                   op0=mult, op1=mult)
                    nc.vector.tensor_scalar(
                        aq_t[:, jg, nsl], v, 224.0, -224.0,
                        op0=mybir.AluOpType.min, op1=mybir.AluOpType.max)

            # ---- mm2: y~^T[h, C] ----
            for m in range(M2):
                w2t = w2pool.tile([128, KT2 * 256], E4, name="w2t", tag="w2")
                nc.sync.dma_start(out=w2t, in_=w2p.ap()[m])
                for nt in range(NTILES):
                    nsl = slice(nt * NT, (nt + 1) * NT)
                    py = psY.tile([128, NT], F32, name="py")
                    for k in range(KT2):
                        lw = w2t[:, k * 256:(k + 1) * 256].rearrange(
                            "p (i m) -> p i m", i=2)
                        ra = aq_t[:, 2 * k:2 * k + 2, nsl]
                        nc.tensor.matmul(py, lw, ra, start=(k == 0),
                                         stop=(k == KT2 - 1), perf_mode=DR)
                    yt = ypool.tile([128, NT], F32, name="yt")
                    nc.vector.tensor_copy(out=yt, in_=py)
                    nc.sync.dma_start(out=yT.ap()[m * 128:(m + 1) * 128, nsl],
                                      in_=yt)
    nc.compile()
    return nc


# --------------------------------------------------------------------------
# Host-side packing
# --------------------------------------------------------------------------

def _halve_to_trn(q_fn_f32):
    """e4m3fn values (held in f32) -> TRN e4m3 at half scale (exact)."""
    return (q_fn_f32.astype(np.float32) * 0.5).astype(E4NP)


def pack_w1(w1_e, h, i_dim):
    """w1_e [2I, H] f32 (e4m3fn values) -> [M1, 128, KT1*256] TRN fp8."""
    M1, KT1 = (2 * i_dim) // 128, h // 256
    q = _halve_to_trn(w1_e)
    t = q.reshape(M1, 128, KT1, 2, 128)            # [m, mm, k, i, p]
    t = np.ascontiguousarray(t.transpose(0, 4, 2, 3, 1))  # [m, p, k, i, mm]
    return t.reshape(M1, 128, KT1 * 256)


def pack_w2(w2_e, h, i_dim):
    """w2_e [H, I] f32 (e4m3fn values) -> [M2, 128, KT2*256] TRN fp8."""
    M2, KT2 = h // 128, i_dim // 256
    q = _halve_to_trn(w2_e)
    t = q.reshape(M2, 128, KT2, 2, 128)
    t = np.ascontiguousarray(t.transpose(0, 4, 2, 3, 1))
    return t.reshape(M2, 128, KT2 * 256)


def quantize_ref(xg, iscale):
    """Exactly the reference's _to_fp8(x/iscale), values in f32."""
    q = np.clip(xg.astype(np.float32) / iscale, -FP8_MAX, FP8_MAX)
    return q.astype(FNNP).astype(np.float32)


def pack_xq(xq_fn_f32, C, h):
    """Quantized tokens [cnt, H] (e4m3fn values) -> [128, 2*KT1, C] TRN fp8."""
    KT1 = h // 256
    cnt = xq_fn_f32.shape[0]
    qt = _halve_to_trn(xq_fn_f32)                  # [cnt, h] TRN fp8
    if cnt < C:
        qt = np.concatenate(
            [qt, np.zeros((C - cnt, h), dtype=E4NP).astype(E4NP)], axis=0)
    xqT = np.ascontiguousarray(qt.T)               # [h, C]
    t = xqT.reshape(KT1, 2, 128, C)                # [k, i, p, n]
    t = np.ascontiguousarray(t.transpose(2, 0, 1, 3))  # [p, k, i, n]
    return t.reshape(128, 2 * KT1, C)


def choose_capacity(max_cnt):
    """Pick token capacity C (multiple of 128) and n-tile size NT <= 512."""
    max_cnt = max(max_cnt, 128)
    n_t = -(-max_cnt // 512)                   # number of n-tiles
    NT = -(-max_cnt // (n_t * 128)) * 128      # per-tile size, mult of 128
    return NT * n_t, NT


# --------------------------------------------------------------------------
# Entry point
# --------------------------------------------------------------------------

def kernel(x, selected_experts, routing_weights, w1, w2,
           w1_iscale, w2_iscale, w1_wscale, w2_wscale):
    global LAST_RESULT
    x = np.asarray(x)
    sel = np.asarray(selected_experts)
    rw = np.asarray(routing_weights).astype(np.float32)
    w1 = np.asarray(w1)
    w2 = np.asarray(w2)
    w1_iscale = np.asarray(w1_iscale, dtype=np.float32)
    w2_iscale = np.asarray(w2_iscale, dtype=np.float32)
    w1_wscale = np.asarray(w1_wscale, dtype=np.float32)
    w2_wscale = np.asarray(w2_wscale, dtype=np.float32)

    t_dim = x.shape[0]
    # gate[t, e] = sum_k rw[t, k] * (sel[t, k] == e)
    gate = np.zeros((t_dim, E), dtype=np.float32)
    rows = np.arange(t_dim)
    for kk in range(sel.shape[1]):
        np.add.at(gate, (rows, sel[:, kk]), rw[:, kk])

    idxs = [np.flatnonzero(gate[:, e] != 0.0) for e in range(E)]
    counts = [len(ix) for ix in idxs]
    C, NT = choose_capacity(max(counts))

    nc = build_nc(C, NT)

    in_maps = []
    for e in range(E):
        ix = idxs[e]
        xq_fn = quantize_ref(x[ix], float(w1_iscale[e]))
        s1 = 4.0 * float(w1_iscale[e]) * float(w1_wscale[e])
        c2 = s1 / (2.0 * float(w2_iscale[e]))
        sc = np.empty((128, 2), dtype=np.float32)
        sc[:, 0] = s1
        sc[:, 1] = c2
        in_maps.append({
            "xq": pack_xq(xq_fn, C, H),
            "w1p": pack_w1(w1[e], H, I),
            "w2p": pack_w2(w2[e], H, I),
            "sc": sc,
        })

    trace = bool(os.environ.get("MOE_TRACE"))
    kwargs = {}
    if trace:
        kwargs["trace"] = True
    res = run_bass_kernel_spmd(nc, in_maps, core_ids=list(range(E)), **kwargs)
    LAST_RESULT = res

    out = np.zeros_like(x, dtype=np.float32)
    for e in range(E):
        ix = idxs[e]
        yTe = res.results[e]["yT"]                   # [H, C] f32 (y~^T)
        ye = np.ascontiguousarray(yTe[:, :len(ix)].T)  # [cnt, H]
        s2 = 4.0 * float(w2_iscale[e]) * float(w2_wscale[e])
        out[ix] += (gate[ix, e] * s2)[:, None] * ye
    return out.astype(x.dtype)
